# revision 1
# baseline (speedup 1.0000x reference)
"""CrossAttention TRN2 Bass kernel — 8-core data-parallel (batch x query-half).

Sharding: core c -> batch b=c//2, query rows [(c%2)*1024, (c%2+1)*1024).
Each core computes its 1024 output rows end-to-end (kv recomputed per
core-pair; no collectives). Host pre-transposes activations so every
matmul operand is contraction-major in DRAM.

kv compaction: masked kv positions are gathered out on the host (pad to
M2C=640 rows of zeros). Zero k rows give exp(0)=1 at pads, but the
ones-column appended to v carries the keep flag, so pads contribute
exactly 0 to both the attention numerator and the softmax denominator —
no mask bias or mask multiply anywhere on-chip.

All matmul operands are bf16, cast on the HOST (ml_dtypes): halves HBM
traffic vs fp32, removes every on-chip weight cast, and draws less PE
power than fp32r (fp32r tripped the HW power throttle harder). PSUM
accumulation is fp32; softmax normalization runs in fp32 via the fast
approx reciprocal (~18 good bits; must read SBUF, not PSUM — probed).
Measured scale-rel error ~4e-3 vs the 2e-2 gate.

Startup: DMA issue costs ~0.65us each on an engine's queue, so the
initial loads are issued from three engines in parallel (weights on
sync, x on gpsimd, y/keep/bias on scalar) — first matmul fires ~9us in.

Phase B is software-pipelined one head deep: scores+exp of head i run
while attn@v + normalize of head i-1 drain, so the exp ACTIVATE has a
full attnv window to free the scores PSUM ring before head i+1 needs it
(the unpipelined version stalled PE 0.8us per head on that ring).
"""

import sys

sys.path.insert(0, "/opt/trn_rl_repo")

from contextlib import ExitStack

import ml_dtypes
import numpy as np

import concourse.bass as bass
import concourse.tile as tile
from concourse import bacc, mybir
from concourse.bass_utils import run_bass_kernel_spmd

B, N, N2 = 4, 2048, 1024
DIM, H, HD = 1024, 16, 64
SCALE = HD ** -0.5
P = 128
R = 1024          # query rows per core
NCORES = 8
KO = DIM // P     # 8 contraction chunks
F32 = mybir.dt.float32
BF = mybir.dt.bfloat16
NPBF = ml_dtypes.bfloat16

M2C = 640         # compacted kv length (5 x 128); kept count must fit
MO = M2C // P

TRACE = False


def _mo_groups(mo):
    """Split mo chunks into groups of 3 then 2 for large ACT ops."""
    groups, i = [], 0
    while mo - i >= 3:
        groups.append((i, 3)); i += 3
    while mo - i > 0:
        g = min(2, mo - i)
        groups.append((i, g)); i += g
    return groups


def build_kernel(m2c=M2C):
    mo_n = m2c // P
    nc = bacc.Bacc("TRN2", target_bir_lowering=False, debug=False,
                   num_devices=NCORES)
    xT = nc.dram_tensor("xT", [DIM, R], BF, kind="ExternalInput").ap()
    yT = nc.dram_tensor("yT", [DIM, m2c], BF, kind="ExternalInput").ap()
    wq = nc.dram_tensor("wq", [DIM, DIM], BF, kind="ExternalInput").ap()
    wk = nc.dram_tensor("wk", [DIM, DIM], BF, kind="ExternalInput").ap()
    wv = nc.dram_tensor("wv", [DIM, DIM], BF, kind="ExternalInput").ap()
    wp = nc.dram_tensor("wp", [DIM, DIM], BF, kind="ExternalInput").ap()
    keepc = nc.dram_tensor("keepc", [m2c], BF, kind="ExternalInput").ap()
    bp = nc.dram_tensor("bp", [DIM], F32, kind="ExternalInput").ap()
    out = nc.dram_tensor("out", [DIM, R], F32, kind="ExternalOutput").ap()

    # kv free-dim chunking for the k^T projection (<=512 for one PSUM bank)
    kv_chunks = [(i, min(320, m2c - i)) for i in range(0, m2c, 320)]

    with tile.TileContext(nc, pool_alloc_mode="queue") as tc, ExitStack() as ctx:
        persist = ctx.enter_context(tc.tile_pool(name="persist", bufs=1))
        qT = persist.tile([P, KO, R], BF)           # q^T, c-major
        kT = persist.tile([P, KO, m2c], BF)         # k^T, c-major
        vS = persist.tile([P, mo_n, H * 65], BF)    # v[m,c] + keep col / head
        attnT = persist.tile([P, KO, R], BF)        # attn out^T, c-major
        wp_r = persist.tile([P, KO, DIM], BF)       # Wproj, loaded during B
        kc = persist.tile([P, mo_n], BF)            # keep col, m-major
        bT = persist.tile([P, KO], F32)

        wq3 = wq.rearrange("(ko p) c -> p ko c", p=P)
        wk3 = wk.rearrange("(ko p) c -> p ko c", p=P)
        wv3 = wv.rearrange("(ko p) c -> p ko c", p=P)
        wp3 = wp.rearrange("(ko p) c -> p ko c", p=P)

        # ---- Phase A: projections. DMA issues spread across engines so
        # the first matmul's inputs are in flight within ~1 issue slot.
        with tc.tile_pool(name="pA", bufs=1) as pa, \
             tc.tile_pool(name="wstg", bufs=2) as wstg, \
             tc.tile_pool(name="psA", bufs=4, space="PSUM") as psa:
            xT_r = pa.tile([P, KO, R], BF)
            yT_r = pa.tile([P, KO, m2c], BF)
            xr3 = xT.rearrange("(ko p) f -> p ko f", p=P)
            yr3 = yT.rearrange("(ko p) f -> p ko f", p=P)

            # sync: first q-weight chunk; gpsimd: x; scalar: keep/bias/y
            wsl0 = wstg.tile([P, KO, P], BF, tag="wq")
            nc.sync.dma_start(wsl0[:], wq3[:, :, 0:P])
            for ko in range(KO):
                nc.gpsimd.dma_start(xT_r[:, ko], xr3[:, ko])
            nc.scalar.dma_start(kc[:], keepc.rearrange("(mo p) -> p mo", p=P))
            nc.scalar.dma_start(bT[:], bp.rearrange("(o p) -> p o", p=P))
            for ko in range(KO):
                nc.scalar.dma_start(yT_r[:, ko], yr3[:, ko])
            vH = vS.rearrange("p mo (h s) -> p mo h s", s=65)
            for mo in range(mo_n):
                nc.vector.tensor_copy(vH[:, mo, :, 64],
                                      kc[:, mo:mo + 1].to_broadcast([P, H]))

            # A1: q = x @ Wq
            for co in range(KO):
                wsl = wsl0 if co == 0 else wstg.tile([P, KO, P], BF, tag="wq")
                if co > 0:
                    nc.sync.dma_start(wsl[:], wq3[:, :, co * P:(co + 1) * P])
                for nn in range(2):
                    ps = psa.tile([P, 512], F32, tag="psa")
                    for ko in range(KO):
                        nc.tensor.matmul(
                            ps[:], wsl[:, ko],
                            xT_r[:, ko, nn * 512:(nn + 1) * 512],
                            start=(ko == 0), stop=(ko == KO - 1))
                    nc.vector.tensor_copy(qT[:, co, nn * 512:(nn + 1) * 512],
                                          ps[:])

            # A2: k^T = Wk^T @ y^T
            for cq in range(4):
                wsl = wstg.tile([P, KO, 256], BF, tag="wk")
                nc.sync.dma_start(wsl[:], wk3[:, :, cq * 256:(cq + 1) * 256])
                for c2 in range(2):
                    co = cq * 2 + c2
                    for m0, mw in kv_chunks:
                        ps = psa.tile([P, 512], F32, tag="psa")
                        for ko in range(KO):
                            nc.tensor.matmul(
                                ps[:, :mw], wsl[:, ko, c2 * P:(c2 + 1) * P],
                                yT_r[:, ko, m0:m0 + mw],
                                start=(ko == 0), stop=(ko == KO - 1))
                        nc.vector.tensor_copy(kT[:, co, m0:m0 + mw],
                                              ps[:, :mw])

            # A3: v = y @ Wv (m-major, heads split, keep col)
            for c4 in range(4):  # 256-wide v column chunks (4 heads)
                wsl = wstg.tile([P, KO, 256], BF, tag="wv")
                nc.sync.dma_start(wsl[:], wv3[:, :, c4 * 256:(c4 + 1) * 256])
                for mo in range(mo_n):
                    ps = psa.tile([P, 512], F32, tag="psa")
                    for ko in range(KO):
                        nc.tensor.matmul(
                            ps[:, 0:256], yT_r[:, ko, mo * P:(mo + 1) * P],
                            wsl[:, ko],
                            start=(ko == 0), stop=(ko == KO - 1))
                    nc.vector.tensor_copy(
                        vH[:, mo, c4 * 4:(c4 + 1) * 4, 0:64],
                        ps[:, 0:256].rearrange("p (h d) -> p h d", d=64))

        # ---- Phase B: attention, software-pipelined one head deep ----
        # Wproj streams in under B's compute shadow (sync is idle here).
        for cw in range(4):
            nc.sync.dma_start(wp_r[:, :, cw * 256:(cw + 1) * 256],
                              wp3[:, :, cw * 256:(cw + 1) * 256])
        groups = _mo_groups(mo_n)
        heads = [(nn, h) for nn in range(2) for h in range(H)]
        with tc.tile_pool(name="pBe", bufs=2) as pbe, \
             tc.tile_pool(name="psS", bufs=2, space="PSUM") as pss, \
             tc.tile_pool(name="psO", bufs=2, space="PSUM") as pso:
            prev = None
            for idx in range(len(heads) + 1):
                cur = None
                if idx < len(heads):
                    nn, h = heads[idx]
                    pq = (h % 2) * 64
                    co = h // 2
                    expS = pbe.tile([P, mo_n, 512], BF, tag="expS")
                    for g0, gn in groups:
                        sps = pss.tile([P, 3, 512], F32, tag="sps")
                        for mo in range(g0, g0 + gn):
                            nc.tensor.matmul(
                                sps[:, mo - g0],
                                kT[pq:pq + 64, co, mo * P:(mo + 1) * P],
                                qT[pq:pq + 64, co, nn * 512:(nn + 1) * 512],
                                start=True, stop=True)
                        nc.scalar.activation(
                            expS[:, g0:g0 + gn], sps[:, :gn],
                            mybir.ActivationFunctionType.Exp,
                            scale=float(SCALE))
                    cur = (expS, nn, h, pq, co)
                if prev is not None:
                    expS0, nn0, h0, pq0, co0 = prev
                    ops = pso.tile([P, 512], F32, tag="ops")
                    for mo in range(mo_n):
                        nc.tensor.matmul(
                            ops[0:65], vS[:, mo, h0 * 65:(h0 + 1) * 65],
                            expS0[:, mo],
                            start=(mo == 0), stop=(mo == mo_n - 1))
                    den = pbe.tile([1, 512], F32, tag="den")
                    nc.scalar.activation(den[:], ops[64:65],
                                         mybir.ActivationFunctionType.Copy)
                    rec = pbe.tile([1, 512], F32, tag="rec")
                    # the custom-DVE recip reads garbage from PSUM, so the
                    # denominator bounces through SBUF (probed on HW)
                    nc.vector.reciprocal_approx_fast(rec[:], den[:])
                    bc = pbe.tile([64, 512], F32, tag="bc")
                    nc.gpsimd.partition_broadcast(bc[:], rec[:])
                    nc.vector.tensor_mul(
                        attnT[pq0:pq0 + 64, co0, nn0 * 512:(nn0 + 1) * 512],
                        ops[0:64], bc[:])
                prev = cur

        # ---- Phase C: outT[c2,n] = Wproj^T-major proj + bias ----
        with tc.tile_pool(name="outp", bufs=3) as outp, \
             tc.tile_pool(name="psC", bufs=4, space="PSUM") as psc:
            for nn in range(2):
                for c2o in range(KO):
                    ps = psc.tile([P, 512], F32, tag="psc")
                    for co in range(KO):
                        nc.tensor.matmul(
                            ps[:], wp_r[:, co, c2o * P:(c2o + 1) * P],
                            attnT[:, co, nn * 512:(nn + 1) * 512],
                            start=(co == 0), stop=(co == KO - 1))
                    osb = outp.tile([P, 512], F32, tag="osb")
                    nc.vector.tensor_scalar_add(osb[:], ps[:], bT[:, c2o:c2o + 1])
                    nc.sync.dma_start(
                        out[c2o * P:(c2o + 1) * P, nn * 512:(nn + 1) * 512], osb[:])

    nc.finalize()
    return nc


_NC = {}


def kernel(x, y, pad_mask, Wq, Wkv, Wproj, bproj):
    x = np.asarray(x, dtype=np.float32)
    y = np.asarray(y, dtype=np.float32)
    pad_mask = np.asarray(pad_mask)
    Wq = np.asarray(Wq, dtype=np.float32)
    Wkv = np.asarray(Wkv, dtype=np.float32)
    Wproj = np.asarray(Wproj, dtype=np.float32)
    bproj = np.asarray(bproj, dtype=np.float32)

    Wqb = np.ascontiguousarray(Wq.astype(NPBF))
    Wkb = np.ascontiguousarray(Wkv[:, :DIM].astype(NPBF))
    Wvb = np.ascontiguousarray(Wkv[:, DIM:].astype(NPBF))
    Wpb = np.ascontiguousarray(Wproj.astype(NPBF))

    # compact kv: gather kept rows per batch, pad with zeros to m2c
    keep_idx = [np.nonzero(pad_mask[b] != 0)[0] for b in range(B)]
    max_kept = max(len(i) for i in keep_idx)
    m2c = M2C if max_kept <= M2C else N2
    yc = np.zeros((B, m2c, DIM), dtype=np.float32)
    keepc = np.zeros((B, m2c), dtype=NPBF)
    for b in range(B):
        k = len(keep_idx[b])
        yc[b, :k] = y[b][keep_idx[b]]
        keepc[b, :k] = 1.0

    xTb = [np.ascontiguousarray(x[b, half * R:(half + 1) * R, :].T.astype(NPBF))
           for b in range(B) for half in range(2)]
    yTb = [np.ascontiguousarray(yc[b].T.astype(NPBF)) for b in range(B)]

    in_maps = []
    for c in range(NCORES):
        b, half = c // 2, c % 2
        in_maps.append({
            "xT": xTb[c],
            "yT": yTb[b],
            "wq": Wqb, "wk": Wkb, "wv": Wvb, "wp": Wpb,
            "keepc": keepc[b],
            "bp": bproj,
        })

    if m2c not in _NC:
        _NC[m2c] = build_kernel(m2c)

    res = run_bass_kernel_spmd(_NC[m2c], in_maps, core_ids=list(range(NCORES)),
                               trace=TRACE)
    if TRACE:
        kernel.last_results = res

    full = np.empty((B, N, DIM), dtype=np.float32)
    for c in range(NCORES):
        b, half = c // 2, c % 2
        full[b, half * R:(half + 1) * R, :] = res.results[c]["out"].T
    return full



# revision 4
# speedup vs baseline: 1.0330x; 1.0330x over previous
"""CrossAttention TRN2 Bass kernel — 8-core data-parallel (batch x query-half).

Sharding: core c -> batch b=c//2, query rows [(c%2)*1024, (c%2+1)*1024).
Each core computes its 1024 output rows end-to-end (kv recomputed per
core-pair; no collectives). Host pre-transposes activations so every
matmul operand is contraction-major in DRAM.

kv compaction: masked kv positions are gathered out on the host (pad to
M2C=640 rows of zeros). Zero k rows give exp(0)=1 at pads, but the
ones-column appended to v carries the keep flag, so pads contribute
exactly 0 to both the attention numerator and the softmax denominator.

v2 restructure vs the 290us baseline:
- Scores matmuls for a head PAIR run row-tiled (64x128 mode, tiles
  T0/T8): head 2co lives on SBUF partitions 0-63, head 2co+1 on 64-127
  (the kT/qT layout already interleaves heads that way), so the two
  64-contraction matmuls execute CONCURRENTLY in the two array halves.
- One fused schedule: per co-slot [A2 kproj(co), A1 qproj(co),
  scores+exp(co), attnv(co-1)] so the ACT-engine exp stream (~11us/pair)
  hides under PE work of the same slot; vproj runs up front with
  y-stationary c4-paired matmuls (80 mms instead of 160).
- exp granularity: one ACT op per m-chunk covering BOTH heads of the
  pair ([P,2,512] PSUM -> [P,2,512] bf16), so PSUM in scores stays at
  3x2 banks and ACT per-op fixed cost stays amortized.
- den handled by the v ones-column as before; den copy moved off ACT
  (vector), normalize mul on vector, partition broadcast on gpsimd.
"""

import sys

sys.path.insert(0, "/opt/trn_rl_repo")

from contextlib import ExitStack

import ml_dtypes
import numpy as np

import concourse.bass as bass
import concourse.tile as tile
from concourse import bacc, mybir
from concourse.bass_utils import run_bass_kernel_spmd

B, N, N2 = 4, 2048, 1024
DIM, H, HD = 1024, 16, 64
SCALE = HD ** -0.5
P = 128
R = 1024          # query rows per core
NCORES = 8
KO = DIM // P     # 8 contraction chunks
F32 = mybir.dt.float32
BF = mybir.dt.bfloat16
NPBF = ml_dtypes.bfloat16

M2C = 640         # compacted kv length (5 x 128); kept count must fit

TRACE = False


def build_kernel(m2c=M2C):
    mo_n = m2c // P
    nc = bacc.Bacc("TRN2", target_bir_lowering=False, debug=False,
                   num_devices=NCORES)
    xT = nc.dram_tensor("xT", [DIM, R], BF, kind="ExternalInput").ap()
    yT = nc.dram_tensor("yT", [DIM, m2c], BF, kind="ExternalInput").ap()
    wq = nc.dram_tensor("wq", [DIM, DIM], BF, kind="ExternalInput").ap()
    wk = nc.dram_tensor("wk", [DIM, DIM], BF, kind="ExternalInput").ap()
    wv = nc.dram_tensor("wv", [DIM, DIM], BF, kind="ExternalInput").ap()
    wp = nc.dram_tensor("wp", [DIM, DIM], BF, kind="ExternalInput").ap()
    keepc = nc.dram_tensor("keepc", [m2c], BF, kind="ExternalInput").ap()
    bp = nc.dram_tensor("bp", [DIM], F32, kind="ExternalInput").ap()
    out = nc.dram_tensor("out", [DIM, R], F32, kind="ExternalOutput").ap()

    # kv free-dim chunking for the k^T projection (<=512 per PSUM bank)
    kv_chunks = [(i, min(512, m2c - i)) for i in range(0, m2c, 512)]
    assert len(kv_chunks) <= 2

    with tile.TileContext(nc, pool_alloc_mode="queue") as tc, ExitStack() as ctx:
        persist = ctx.enter_context(tc.tile_pool(name="persist", bufs=1))
        qT = persist.tile([P, KO, R], BF)           # q^T, c-major
        kT = persist.tile([P, KO, m2c], BF)         # k^T, c-major
        vS = persist.tile([P, mo_n, H * 65], BF)    # v[m,c] + keep col / head
        attnT = persist.tile([P, KO, R], BF)        # attn out^T, c-major
        wp_r = persist.tile([P, KO, DIM], BF)       # Wproj, loaded during B
        kc = persist.tile([P, mo_n], BF)            # keep col, m-major
        bT = persist.tile([P, KO], F32)
        xT_r = persist.tile([P, KO, R], BF)
        yT_r = persist.tile([P, KO, m2c], BF)
        wv_t = persist.tile([P, KO, DIM], BF)       # full Wv staged

        wq3 = wq.rearrange("(ko p) c -> p ko c", p=P)
        wk3 = wk.rearrange("(ko p) c -> p ko c", p=P)
        wv3 = wv.rearrange("(ko p) c -> p ko c", p=P)
        wp3 = wp.rearrange("(ko p) c -> p ko c", p=P)
        xr3 = xT.rearrange("(ko p) f -> p ko f", p=P)
        yr3 = yT.rearrange("(ko p) f -> p ko f", p=P)

        wstg = ctx.enter_context(tc.tile_pool(name="wstg", bufs=2))
        pbe = ctx.enter_context(tc.tile_pool(name="pbe", bufs=2))
        outp = ctx.enter_context(tc.tile_pool(name="outp", bufs=3))
        # PSUM: ps2 3x[P,2,512] (6 banks) + psv 2x[P,512] (2 banks) = 8
        psS = ctx.enter_context(tc.tile_pool(name="psS", bufs=3, space="PSUM"))
        psV = ctx.enter_context(tc.tile_pool(name="psV", bufs=2, space="PSUM"))

        # ---- initial DMA issues, spread across engine queues ----
        nc.scalar.dma_start(kc[:], keepc.rearrange("(mo p) -> p mo", p=P))
        nc.scalar.dma_start(bT[:], bp.rearrange("(o p) -> p o", p=P))
        for ko in range(KO):
            nc.scalar.dma_start(yT_r[:, ko], yr3[:, ko])
        wk_t = wstg.tile([P, KO, 256], BF, tag="wk")
        nc.sync.dma_start(wk_t[:], wk3[:, :, 0:256])
        for i in range(4):
            nc.sync.dma_start(wv_t[:, :, i * 256:(i + 1) * 256],
                              wv3[:, :, i * 256:(i + 1) * 256])
        wq_t = wstg.tile([P, KO, P], BF, tag="wq")
        nc.sync.dma_start(wq_t[:], wq3[:, :, 0:P])
        for ko in range(KO):
            nc.gpsimd.dma_start(xT_r[:, ko], xr3[:, ko])
        vH = vS.rearrange("p mo (h s) -> p mo h s", s=65)
        for mo in range(mo_n):
            nc.gpsimd.tensor_copy(vH[:, mo, :, 64],
                                  kc[:, mo:mo + 1].to_broadcast([P, H]))

        def a2_kproj(co, wk_cur):
            """kT[:, co] <- Wk[:, co-block]^T @ y^T (8 ko accumulation)."""
            c2 = co % 2
            psk = psS.tile([P, 2, 512], F32, tag="ps2")
            for ko in range(KO):
                for ci, (m0, mw) in enumerate(kv_chunks):
                    nc.tensor.matmul(
                        psk[:, ci, :mw], wk_cur[:, ko, c2 * P:(c2 + 1) * P],
                        yT_r[:, ko, m0:m0 + mw],
                        start=(ko == 0), stop=(ko == KO - 1))
            for ci, (m0, mw) in enumerate(kv_chunks):
                nc.vector.tensor_copy(kT[:, co, m0:m0 + mw], psk[:, ci, :mw])

        def a1_qproj(co, wq_cur):
            """qT[:, co] <- x @ Wq[:, co-block], both query halves."""
            psq = psS.tile([P, 2, 512], F32, tag="ps2")
            for ko in range(KO):
                for nn2 in range(2):
                    nc.tensor.matmul(
                        psq[:, nn2], wq_cur[:, ko],
                        xT_r[:, ko, nn2 * 512:(nn2 + 1) * 512],
                        start=(ko == 0), stop=(ko == KO - 1))
            nc.vector.tensor_copy(qT[:, co, :], psq[:, :, :])

        def a3_vproj():
            """v = y @ Wv, m-major, y-stationary, c4-paired (N=512)."""
            for mo in range(mo_n):
                psv = psS.tile([P, 2, 512], F32, tag="ps2")
                for ko in range(KO):
                    for cp in range(2):
                        nc.tensor.matmul(
                            psv[:, cp], yT_r[:, ko, mo * P:(mo + 1) * P],
                            wv_t[:, ko, cp * 512:(cp + 1) * 512],
                            start=(ko == 0), stop=(ko == KO - 1))
                nc.vector.tensor_copy(
                    vH[:, mo, :, 0:64],
                    psv[:, :, :].rearrange("p c2 (h d) -> p (c2 h) d", d=64))

        def scores_exp(co, nn2):
            """Row-tiled scores for head pair (2co, 2co+1) + exp."""
            ex = pbe.tile([P, mo_n, 2, 512], BF, tag="expS", bufs=4)
            for c in range(mo_n):
                pss = psS.tile([P, 2, 512], F32, tag="ps2")
                nc.tensor.matmul(
                    pss[:, 0], kT[0:64, co, c * P:(c + 1) * P],
                    qT[0:64, co, nn2 * 512:(nn2 + 1) * 512],
                    start=True, stop=True)
                nc.tensor.matmul(
                    pss[:, 1], kT[64:128, co, c * P:(c + 1) * P],
                    qT[64:128, co, nn2 * 512:(nn2 + 1) * 512],
                    start=True, stop=True)
                nc.scalar.activation(
                    ex[:, c], pss[:, :, :],
                    mybir.ActivationFunctionType.Exp, scale=float(SCALE))
            return ex

        def attnv_norm(co, exs):
            """attn @ v + softmax normalize for the pair's 4 instances."""
            for nn2 in range(2):
                ex = exs[nn2]
                for h01 in range(2):
                    h = 2 * co + h01
                    ops = psV.tile([P, 512], F32, tag="psv")
                    for c in range(mo_n):
                        nc.tensor.matmul(
                            ops[0:65], vS[:, c, h * 65:(h + 1) * 65],
                            ex[:, c, h01],
                            start=(c == 0), stop=(c == mo_n - 1))
                    den = pbe.tile([1, 512], F32, tag="den")
                    nc.vector.tensor_copy(den[:], ops[64:65])
                    rec = pbe.tile([1, 512], F32, tag="rec")
                    # approx recip must read SBUF, not PSUM (probed on HW)
                    nc.vector.reciprocal_approx_fast(rec[:], den[:])
                    bc = pbe.tile([64, 512], F32, tag="bc")
                    nc.gpsimd.partition_broadcast(bc[:], rec[:])
                    nc.vector.tensor_mul(
                        attnT[h01 * 64:h01 * 64 + 64, co,
                              nn2 * 512:(nn2 + 1) * 512],
                        ops[0:64], bc[:])

        # ---- fused schedule ----
        a2_kproj(0, wk_t)
        a3_vproj()
        prev = None
        for co in range(KO):
            cq = co // 2
            if co % 2 == 0 and cq < 3:      # prefetch wk chunk cq+1
                wk_nxt = wstg.tile([P, KO, 256], BF, tag="wk")
                nc.sync.dma_start(wk_nxt[:],
                                  wk3[:, :, (cq + 1) * 256:(cq + 2) * 256])
            if co < KO - 1:                  # prefetch wq chunk co+1
                wq_nxt = wstg.tile([P, KO, P], BF, tag="wq")
                nc.sync.dma_start(wq_nxt[:],
                                  wq3[:, :, (co + 1) * P:(co + 2) * P])
            if co >= 4:                      # stream Wproj under B's shadow
                cw = co - 4
                nc.sync.dma_start(wp_r[:, :, cw * 256:(cw + 1) * 256],
                                  wp3[:, :, cw * 256:(cw + 1) * 256])
            if co > 0:
                if co % 2 == 0:
                    wk_t = wk_nxt_used
                a2_kproj(co, wk_t)
            if co % 2 == 0 and cq < 3:
                wk_nxt_used = wk_nxt
            a1_qproj(co, wq_t)
            if co < KO - 1:
                wq_t = wq_nxt
            exs = (scores_exp(co, 0), scores_exp(co, 1))
            if prev is not None:
                attnv_norm(prev[0], prev[1])
            prev = (co, exs)
        attnv_norm(prev[0], prev[1])

        # ---- Phase C: outT[c2,n] = Wproj^T @ attnT + bias ----
        for c2o in range(KO):
            psc = [psV.tile([P, 512], F32, tag="psv", name=f"psc{nn2}")
                   for nn2 in range(2)]
            for co in range(KO):
                for nn2 in range(2):
                    nc.tensor.matmul(
                        psc[nn2][:], wp_r[:, co, c2o * P:(c2o + 1) * P],
                        attnT[:, co, nn2 * 512:(nn2 + 1) * 512],
                        start=(co == 0), stop=(co == KO - 1))
            for nn2 in range(2):
                osb = outp.tile([P, 512], F32, tag="osb")
                nc.vector.tensor_scalar_add(osb[:], psc[nn2][:],
                                            bT[:, c2o:c2o + 1])
                nc.sync.dma_start(
                    out[c2o * P:(c2o + 1) * P, nn2 * 512:(nn2 + 1) * 512],
                    osb[:])

    nc.finalize()
    return nc


_NC = {}


def kernel(x, y, pad_mask, Wq, Wkv, Wproj, bproj):
    x = np.asarray(x, dtype=np.float32)
    y = np.asarray(y, dtype=np.float32)
    pad_mask = np.asarray(pad_mask)
    Wq = np.asarray(Wq, dtype=np.float32)
    Wkv = np.asarray(Wkv, dtype=np.float32)
    Wproj = np.asarray(Wproj, dtype=np.float32)
    bproj = np.asarray(bproj, dtype=np.float32)

    Wqb = np.ascontiguousarray(Wq.astype(NPBF))
    Wkb = np.ascontiguousarray(Wkv[:, :DIM].astype(NPBF))
    Wvb = np.ascontiguousarray(Wkv[:, DIM:].astype(NPBF))
    Wpb = np.ascontiguousarray(Wproj.astype(NPBF))

    # compact kv: gather kept rows per batch, pad with zeros to m2c
    keep_idx = [np.nonzero(pad_mask[b] != 0)[0] for b in range(B)]
    max_kept = max(len(i) for i in keep_idx)
    m2c = M2C if max_kept <= M2C else N2
    yc = np.zeros((B, m2c, DIM), dtype=np.float32)
    keepc = np.zeros((B, m2c), dtype=NPBF)
    for b in range(B):
        k = len(keep_idx[b])
        yc[b, :k] = y[b][keep_idx[b]]
        keepc[b, :k] = 1.0

    xTb = [np.ascontiguousarray(x[b, half * R:(half + 1) * R, :].T.astype(NPBF))
           for b in range(B) for half in range(2)]
    yTb = [np.ascontiguousarray(yc[b].T.astype(NPBF)) for b in range(B)]

    in_maps = []
    for c in range(NCORES):
        b, half = c // 2, c % 2
        in_maps.append({
            "xT": xTb[c],
            "yT": yTb[b],
            "wq": Wqb, "wk": Wkb, "wv": Wvb, "wp": Wpb,
            "keepc": keepc[b],
            "bp": bproj,
        })

    if m2c not in _NC:
        _NC[m2c] = build_kernel(m2c)

    res = run_bass_kernel_spmd(_NC[m2c], in_maps, core_ids=list(range(NCORES)),
                               trace=TRACE)
    if TRACE:
        kernel.last_results = res

    full = np.empty((B, N, DIM), dtype=np.float32)
    for c in range(NCORES):
        b, half = c // 2, c % 2
        full[b, half * R:(half + 1) * R, :] = res.results[c]["out"].T
    return full


# revision 6
# speedup vs baseline: 1.0456x; 1.0121x over previous
"""CrossAttention TRN2 Bass kernel — 8-core data-parallel (batch x query-half).

Sharding: core c -> batch b=c//2, query rows [(c%2)*1024, (c%2+1)*1024).
Each core computes its 1024 output rows end-to-end (kv recomputed per
core-pair; no collectives). Host pre-transposes activations so every
matmul operand is contraction-major in DRAM.

kv compaction: masked kv positions are gathered out on the host (pad to
M2C=640 rows of zeros). Zero k rows give exp(0)=1 at pads, but the
ones-column appended to v carries the keep flag, so pads contribute
exactly 0 to both the attention numerator and the softmax denominator.

v2 restructure vs the 290us baseline:
- Scores matmuls for a head PAIR run row-tiled (64x128 mode, tiles
  T0/T8): head 2co lives on SBUF partitions 0-63, head 2co+1 on 64-127
  (the kT/qT layout already interleaves heads that way), so the two
  64-contraction matmuls execute CONCURRENTLY in the two array halves.
- One fused schedule: per co-slot [A2 kproj(co), A1 qproj(co),
  scores+exp(co), attnv(co-1)] so the ACT-engine exp stream (~11us/pair)
  hides under PE work of the same slot; vproj runs up front with
  y-stationary c4-paired matmuls (80 mms instead of 160).
- exp granularity: one ACT op per m-chunk covering BOTH heads of the
  pair ([P,2,512] PSUM -> [P,2,512] bf16), so PSUM in scores stays at
  3x2 banks and ACT per-op fixed cost stays amortized.
- den handled by the v ones-column as before; den copy moved off ACT
  (vector), normalize mul on vector, partition broadcast on gpsimd.
"""

import sys

sys.path.insert(0, "/opt/trn_rl_repo")

from contextlib import ExitStack

import ml_dtypes
import numpy as np

import concourse.bass as bass
import concourse.tile as tile
from concourse import bacc, mybir
from concourse.bass_utils import run_bass_kernel_spmd

B, N, N2 = 4, 2048, 1024
DIM, H, HD = 1024, 16, 64
SCALE = HD ** -0.5
P = 128
R = 1024          # query rows per core
NCORES = 8
KO = DIM // P     # 8 contraction chunks
F32 = mybir.dt.float32
BF = mybir.dt.bfloat16
NPBF = ml_dtypes.bfloat16

M2C = 640         # compacted kv length (5 x 128); kept count must fit

TRACE = False


def build_kernel(m2c=M2C):
    mo_n = m2c // P
    nc = bacc.Bacc("TRN2", target_bir_lowering=False, debug=False,
                   num_devices=NCORES)
    xT = nc.dram_tensor("xT", [DIM, R], BF, kind="ExternalInput").ap()
    yT = nc.dram_tensor("yT", [DIM, m2c], BF, kind="ExternalInput").ap()
    wq = nc.dram_tensor("wq", [DIM, DIM], BF, kind="ExternalInput").ap()
    wk = nc.dram_tensor("wk", [DIM, DIM], BF, kind="ExternalInput").ap()
    wv = nc.dram_tensor("wv", [DIM, DIM], BF, kind="ExternalInput").ap()
    wp = nc.dram_tensor("wp", [DIM, DIM], BF, kind="ExternalInput").ap()
    keepc = nc.dram_tensor("keepc", [m2c], BF, kind="ExternalInput").ap()
    bp = nc.dram_tensor("bp", [DIM], F32, kind="ExternalInput").ap()
    out = nc.dram_tensor("out", [DIM, R], F32, kind="ExternalOutput").ap()

    # kv free-dim chunking for the k^T projection (<=512 per PSUM bank)
    kv_chunks = [(i, min(512, m2c - i)) for i in range(0, m2c, 512)]
    assert len(kv_chunks) <= 2

    with tile.TileContext(nc, pool_alloc_mode="queue") as tc, ExitStack() as ctx:
        persist = ctx.enter_context(tc.tile_pool(name="persist", bufs=1))
        qT = persist.tile([P, KO, R], BF)           # q^T, c-major
        kT = persist.tile([P, KO, m2c], BF)         # k^T, c-major
        vS = persist.tile([P, mo_n, H * 65], BF)    # v[m,c] + keep col / head
        attnT = persist.tile([P, KO, R], BF)        # attn out^T, c-major
        wp_r = persist.tile([P, KO, DIM], BF)       # Wproj, loaded during B
        kc = persist.tile([P, mo_n], BF)            # keep col, m-major
        bT = persist.tile([P, KO], F32)
        xT_r = persist.tile([P, KO, R], BF)
        yT_r = persist.tile([P, KO, m2c], BF)
        wv_t = persist.tile([P, KO, DIM], BF)       # full Wv staged

        wq3 = wq.rearrange("(ko p) c -> p ko c", p=P)
        wk3 = wk.rearrange("(ko p) c -> p ko c", p=P)
        wv3 = wv.rearrange("(ko p) c -> p ko c", p=P)
        wp3 = wp.rearrange("(ko p) c -> p ko c", p=P)
        xr3 = xT.rearrange("(ko p) f -> p ko f", p=P)
        yr3 = yT.rearrange("(ko p) f -> p ko f", p=P)

        wstg = ctx.enter_context(tc.tile_pool(name="wstg", bufs=2))
        pbe = ctx.enter_context(tc.tile_pool(name="pbe", bufs=2))
        outp = ctx.enter_context(tc.tile_pool(name="outp", bufs=3))
        # PSUM: ps2 3x[P,2,512] (6 banks) + psv 2x[P,512] (2 banks) = 8
        psS = ctx.enter_context(tc.tile_pool(name="psS", bufs=3, space="PSUM"))
        psV = ctx.enter_context(tc.tile_pool(name="psV", bufs=2, space="PSUM"))

        # ---- initial DMA issues, ordered by when compute needs them:
        # y+wk0 gate A2(0) (~5us), x+wq0 gate A1(0) (~12us), wv gates A3.
        for ko in range(KO):
            nc.scalar.dma_start(yT_r[:, ko], yr3[:, ko])
        nc.scalar.dma_start(kc[:], keepc.rearrange("(mo p) -> p mo", p=P))
        nc.scalar.dma_start(bT[:], bp.rearrange("(o p) -> p o", p=P))
        wk_t = wstg.tile([P, KO, 256], BF, tag="wk")
        nc.sync.dma_start(wk_t[:], wk3[:, :, 0:256])
        wq_t = wstg.tile([P, KO, P], BF, tag="wq")
        nc.sync.dma_start(wq_t[:], wq3[:, :, 0:P])
        for i in range(4):
            nc.sync.dma_start(wv_t[:, :, i * 256:(i + 1) * 256],
                              wv3[:, :, i * 256:(i + 1) * 256])
        for ko in range(KO):
            nc.gpsimd.dma_start(xT_r[:, ko], xr3[:, ko])
        vH = vS.rearrange("p mo (h s) -> p mo h s", s=65)
        for mo in range(mo_n):
            nc.gpsimd.tensor_copy(vH[:, mo, :, 64],
                                  kc[:, mo:mo + 1].to_broadcast([P, H]))

        def a2_kproj(co, wk_cur):
            """kT[:, co] <- Wk[:, co-block]^T @ y^T (8 ko accumulation)."""
            c2 = co % 2
            psk = psS.tile([P, 2, 512], F32, tag="ps2")
            for ko in range(KO):
                for ci, (m0, mw) in enumerate(kv_chunks):
                    nc.tensor.matmul(
                        psk[:, ci, :mw], wk_cur[:, ko, c2 * P:(c2 + 1) * P],
                        yT_r[:, ko, m0:m0 + mw],
                        start=(ko == 0), stop=(ko == KO - 1))
            for ci, (m0, mw) in enumerate(kv_chunks):
                nc.vector.tensor_copy(kT[:, co, m0:m0 + mw], psk[:, ci, :mw])

        def a1_qproj(co, wq_cur):
            """qT[:, co] <- x @ Wq[:, co-block], both query halves."""
            psq = psS.tile([P, 2, 512], F32, tag="ps2")
            for ko in range(KO):
                for nn2 in range(2):
                    nc.tensor.matmul(
                        psq[:, nn2], wq_cur[:, ko],
                        xT_r[:, ko, nn2 * 512:(nn2 + 1) * 512],
                        start=(ko == 0), stop=(ko == KO - 1))
            nc.vector.tensor_copy(qT[:, co, :], psq[:, :, :])

        def a3_vproj():
            """v = y @ Wv, m-major, y-stationary, c4-paired (N=512)."""
            for mo in range(mo_n):
                psv = psS.tile([P, 2, 512], F32, tag="ps2")
                for ko in range(KO):
                    for cp in range(2):
                        nc.tensor.matmul(
                            psv[:, cp], yT_r[:, ko, mo * P:(mo + 1) * P],
                            wv_t[:, ko, cp * 512:(cp + 1) * 512],
                            start=(ko == 0), stop=(ko == KO - 1))
                nc.vector.tensor_copy(
                    vH[:, mo, :, 0:64],
                    psv[:, :, :].rearrange("p c2 (h d) -> p (c2 h) d", d=64))

        def scores_exp(co, nn2):
            """Row-tiled scores for head pair (2co, 2co+1) + exp."""
            ex = pbe.tile([P, mo_n, 2, 512], BF, tag="expS", bufs=4)
            for c in range(mo_n):
                pss = psS.tile([P, 2, 512], F32, tag="ps2")
                nc.tensor.matmul(
                    pss[:, 0], kT[0:64, co, c * P:(c + 1) * P],
                    qT[0:64, co, nn2 * 512:(nn2 + 1) * 512],
                    start=True, stop=True)
                nc.tensor.matmul(
                    pss[:, 1], kT[64:128, co, c * P:(c + 1) * P],
                    qT[64:128, co, nn2 * 512:(nn2 + 1) * 512],
                    start=True, stop=True)
                nc.scalar.activation(
                    ex[:, c], pss[:, :, :],
                    mybir.ActivationFunctionType.Exp, scale=float(SCALE))
            return ex

        def attnv_norm(co, exs):
            """attn @ v + softmax normalize for the pair's 4 instances."""
            for nn2 in range(2):
                ex = exs[nn2]
                for h01 in range(2):
                    h = 2 * co + h01
                    ops = psV.tile([P, 512], F32, tag="psv")
                    for c in range(mo_n):
                        nc.tensor.matmul(
                            ops[0:65], vS[:, c, h * 65:(h + 1) * 65],
                            ex[:, c, h01],
                            start=(c == 0), stop=(c == mo_n - 1))
                    den = pbe.tile([1, 512], F32, tag="den")
                    nc.vector.tensor_copy(den[:], ops[64:65])
                    rec = pbe.tile([1, 512], F32, tag="rec")
                    # approx recip must read SBUF, not PSUM (probed on HW)
                    nc.vector.reciprocal_approx_fast(rec[:], den[:])
                    bc = pbe.tile([64, 512], F32, tag="bc")
                    nc.gpsimd.partition_broadcast(bc[:], rec[:])
                    nc.vector.tensor_mul(
                        attnT[h01 * 64:h01 * 64 + 64, co,
                              nn2 * 512:(nn2 + 1) * 512],
                        ops[0:64], bc[:])

        # ---- fused schedule; A2/A1 run one slot ahead of scores so the
        # scores ldweights never wait on the just-issued qT/kT evacuation.
        wk_tiles = {0: wk_t}

        def a2_sched(co):
            """Run A2(co), prefetching the wk chunk for co+2 first."""
            cq = co // 2
            if co % 2 == 0 and cq + 1 < 4:
                wk_nxt = wstg.tile([P, KO, 256], BF, tag="wk")
                nc.sync.dma_start(wk_nxt[:],
                                  wk3[:, :, (cq + 1) * 256:(cq + 2) * 256])
                wk_tiles[cq + 1] = wk_nxt
            a2_kproj(co, wk_tiles[cq])

        def a1_sched(co):
            nonlocal wq_t
            if co + 1 < KO:
                wq_nxt = wstg.tile([P, KO, P], BF, tag="wq")
                nc.sync.dma_start(wq_nxt[:],
                                  wq3[:, :, (co + 1) * P:(co + 2) * P])
            a1_qproj(co, wq_t)
            if co + 1 < KO:
                wq_t = wq_nxt

        a2_sched(0)
        a2_sched(1)
        a1_sched(0)
        prev = None
        for co in range(KO):
            if co + 2 < KO:
                a2_sched(co + 2)
            if co + 1 < KO:
                a1_sched(co + 1)
            if co >= 4:                      # stream Wproj under B's shadow
                cw = co - 4
                nc.sync.dma_start(wp_r[:, :, cw * 256:(cw + 1) * 256],
                                  wp3[:, :, cw * 256:(cw + 1) * 256])
            exs = (scores_exp(co, 0), scores_exp(co, 1))
            if co == 0:
                a3_vproj()
            if prev is not None:
                attnv_norm(prev[0], prev[1])
            prev = (co, exs)
        attnv_norm(prev[0], prev[1])

        # ---- Phase C: outT[c2,n] = Wproj^T @ attnT + bias ----
        for c2o in range(KO):
            psc = [psV.tile([P, 512], F32, tag="psv", name=f"psc{nn2}")
                   for nn2 in range(2)]
            for co in range(KO):
                for nn2 in range(2):
                    nc.tensor.matmul(
                        psc[nn2][:], wp_r[:, co, c2o * P:(c2o + 1) * P],
                        attnT[:, co, nn2 * 512:(nn2 + 1) * 512],
                        start=(co == 0), stop=(co == KO - 1))
            for nn2 in range(2):
                osb = outp.tile([P, 512], F32, tag="osb")
                nc.vector.tensor_scalar_add(osb[:], psc[nn2][:],
                                            bT[:, c2o:c2o + 1])
                nc.sync.dma_start(
                    out[c2o * P:(c2o + 1) * P, nn2 * 512:(nn2 + 1) * 512],
                    osb[:])

    nc.finalize()
    return nc


_NC = {}


def kernel(x, y, pad_mask, Wq, Wkv, Wproj, bproj):
    x = np.asarray(x, dtype=np.float32)
    y = np.asarray(y, dtype=np.float32)
    pad_mask = np.asarray(pad_mask)
    Wq = np.asarray(Wq, dtype=np.float32)
    Wkv = np.asarray(Wkv, dtype=np.float32)
    Wproj = np.asarray(Wproj, dtype=np.float32)
    bproj = np.asarray(bproj, dtype=np.float32)

    Wqb = np.ascontiguousarray(Wq.astype(NPBF))
    Wkb = np.ascontiguousarray(Wkv[:, :DIM].astype(NPBF))
    Wvb = np.ascontiguousarray(Wkv[:, DIM:].astype(NPBF))
    Wpb = np.ascontiguousarray(Wproj.astype(NPBF))

    # compact kv: gather kept rows per batch, pad with zeros to m2c
    keep_idx = [np.nonzero(pad_mask[b] != 0)[0] for b in range(B)]
    max_kept = max(len(i) for i in keep_idx)
    m2c = M2C if max_kept <= M2C else N2
    yc = np.zeros((B, m2c, DIM), dtype=np.float32)
    keepc = np.zeros((B, m2c), dtype=NPBF)
    for b in range(B):
        k = len(keep_idx[b])
        yc[b, :k] = y[b][keep_idx[b]]
        keepc[b, :k] = 1.0

    xTb = [np.ascontiguousarray(x[b, half * R:(half + 1) * R, :].T.astype(NPBF))
           for b in range(B) for half in range(2)]
    yTb = [np.ascontiguousarray(yc[b].T.astype(NPBF)) for b in range(B)]

    in_maps = []
    for c in range(NCORES):
        b, half = c // 2, c % 2
        in_maps.append({
            "xT": xTb[c],
            "yT": yTb[b],
            "wq": Wqb, "wk": Wkb, "wv": Wvb, "wp": Wpb,
            "keepc": keepc[b],
            "bp": bproj,
        })

    if m2c not in _NC:
        _NC[m2c] = build_kernel(m2c)

    res = run_bass_kernel_spmd(_NC[m2c], in_maps, core_ids=list(range(NCORES)),
                               trace=TRACE)
    if TRACE:
        kernel.last_results = res

    full = np.empty((B, N, DIM), dtype=np.float32)
    for c in range(NCORES):
        b, half = c // 2, c % 2
        full[b, half * R:(half + 1) * R, :] = res.results[c]["out"].T
    return full


# revision 12
# speedup vs baseline: 1.0624x; 1.0161x over previous
"""CrossAttention TRN2 Bass kernel — 8-core data-parallel (batch x query-half).

Sharding: core c -> batch b=c//2, query rows [(c%2)*1024, (c%2+1)*1024).
Each core computes its 1024 output rows end-to-end (kv recomputed per
core-pair; no collectives). Host pre-transposes activations so every
matmul operand is contraction-major in DRAM.

kv compaction: masked kv positions are gathered out on the host (pad to
M2C=640 rows of zeros). Zero k rows give exp(0)=1 at pads, but the
ones-column appended to v carries the keep flag, so pads contribute
exactly 0 to both the attention numerator and the softmax denominator.

v2 restructure vs the 290us baseline:
- Scores matmuls for a head PAIR run row-tiled (64x128 mode, tiles
  T0/T8): head 2co lives on SBUF partitions 0-63, head 2co+1 on 64-127
  (the kT/qT layout already interleaves heads that way), so the two
  64-contraction matmuls execute CONCURRENTLY in the two array halves.
- One fused schedule: per co-slot [A2 kproj(co), A1 qproj(co),
  scores+exp(co), attnv(co-1)] so the ACT-engine exp stream (~11us/pair)
  hides under PE work of the same slot; vproj runs up front with
  y-stationary c4-paired matmuls (80 mms instead of 160).
- exp granularity: one ACT op per m-chunk covering BOTH heads of the
  pair ([P,2,512] PSUM -> [P,2,512] bf16), so PSUM in scores stays at
  3x2 banks and ACT per-op fixed cost stays amortized.
- den handled by the v ones-column as before; den copy moved off ACT
  (vector), normalize mul on vector, partition broadcast on gpsimd.
"""

import sys

sys.path.insert(0, "/opt/trn_rl_repo")

from contextlib import ExitStack

import ml_dtypes
import numpy as np

import concourse.bass as bass
import concourse.tile as tile
from concourse import bacc, mybir
from concourse.bass_utils import run_bass_kernel_spmd

B, N, N2 = 4, 2048, 1024
DIM, H, HD = 1024, 16, 64
SCALE = HD ** -0.5
P = 128
R = 1024          # query rows per core
NCORES = 8
KO = DIM // P     # 8 contraction chunks
F32 = mybir.dt.float32
BF = mybir.dt.bfloat16
NPBF = ml_dtypes.bfloat16

M2C = 640         # compacted kv length (5 x 128); kept count must fit

TRACE = False


def build_kernel(m2c=M2C):
    mo_n = m2c // P
    nc = bacc.Bacc("TRN2", target_bir_lowering=False, debug=False,
                   num_devices=NCORES)
    xT = nc.dram_tensor("xT", [DIM, R], BF, kind="ExternalInput").ap()
    yT = nc.dram_tensor("yT", [DIM, m2c], BF, kind="ExternalInput").ap()
    wq = nc.dram_tensor("wq", [DIM, DIM], BF, kind="ExternalInput").ap()
    wk = nc.dram_tensor("wk", [DIM, DIM], BF, kind="ExternalInput").ap()
    wv = nc.dram_tensor("wv", [DIM, DIM], BF, kind="ExternalInput").ap()
    wp = nc.dram_tensor("wp", [DIM, DIM], BF, kind="ExternalInput").ap()
    keepc = nc.dram_tensor("keepc", [m2c], BF, kind="ExternalInput").ap()
    bp = nc.dram_tensor("bp", [DIM], F32, kind="ExternalInput").ap()
    out = nc.dram_tensor("out", [DIM, R], F32, kind="ExternalOutput").ap()

    # kv free-dim chunking for the k^T projection (<=512 per PSUM bank)
    kv_chunks = [(i, min(512, m2c - i)) for i in range(0, m2c, 512)]
    assert len(kv_chunks) <= 2

    with tile.TileContext(nc, pool_alloc_mode="queue") as tc, ExitStack() as ctx:
        persist = ctx.enter_context(tc.tile_pool(name="persist", bufs=1))
        qT = persist.tile([P, KO, R], BF)           # q^T, c-major
        kT = persist.tile([P, KO, m2c], BF)         # k^T, c-major
        vS = persist.tile([P, mo_n, H * 65], BF)    # v[m,c] + keep col / head
        attnT = persist.tile([P, KO, R], BF)        # attn out^T, c-major
        wp_r = persist.tile([P, KO, DIM], BF)       # Wproj, loaded during B
        kc = persist.tile([P, mo_n], BF)            # keep col, m-major
        bT = persist.tile([P, KO], F32)
        xT_r = persist.tile([P, KO, R], BF)
        yT_r = persist.tile([P, KO, m2c], BF)
        wv_t = persist.tile([P, KO, DIM], BF)       # full Wv staged

        wq3 = wq.rearrange("(ko p) c -> p ko c", p=P)
        wk3 = wk.rearrange("(ko p) c -> p ko c", p=P)
        wv3 = wv.rearrange("(ko p) c -> p ko c", p=P)
        wp3 = wp.rearrange("(ko p) c -> p ko c", p=P)
        xr3 = xT.rearrange("(ko p) f -> p ko f", p=P)
        yr3 = yT.rearrange("(ko p) f -> p ko f", p=P)

        wstg = ctx.enter_context(tc.tile_pool(name="wstg", bufs=2))
        pbe = ctx.enter_context(tc.tile_pool(name="pbe", bufs=2))
        outp = ctx.enter_context(tc.tile_pool(name="outp", bufs=3))
        # PSUM: ps2 3x[P,2,512] (6 banks) + psv 2x[P,512] (2 banks) = 8
        psS = ctx.enter_context(tc.tile_pool(name="psS", bufs=3, space="PSUM"))
        psV = ctx.enter_context(tc.tile_pool(name="psV", bufs=2, space="PSUM"))

        # ---- initial DMA issues, ordered by when compute needs them:
        # y+wk0 gate A2(0) (~5us), x+wq0 gate A1(0) (~12us), wv gates A3.
        # Only A2(0)'s inputs are issued before its emission so its DMA
        # watermark stays low (deps are per-queue counters).
        for ko in range(KO):
            nc.scalar.dma_start(yT_r[:, ko], yr3[:, ko])
        wk_t = wstg.tile([P, KO, 256], BF, tag="wk")
        nc.sync.dma_start(wk_t[:], wk3[:, :, 0:256])

        # PE warmup: ~5us of throwaway matmuls so the tensor clock is at
        # max p-state by the time the real work lands.
        warm = persist.tile([P, 512], BF)
        nc.vector.memset(warm[:], 0)
        psw = psV.tile([P, 512], F32, tag="psv", name="psw")
        for i in range(24):
            nc.tensor.matmul(psw[0:64, :], warm[:, 0:64], warm[:, :],
                             start=True, stop=True)

        def a2_kproj(co, wk_cur):
            """kT[:, co] <- Wk[:, co-block]^T @ y^T (8 ko accumulation)."""
            c2 = co % 2
            psk = psS.tile([P, 2, 512], F32, tag="ps2")
            for ko in range(KO):
                for ci, (m0, mw) in enumerate(kv_chunks):
                    nc.tensor.matmul(
                        psk[:, ci, :mw], wk_cur[:, ko, c2 * P:(c2 + 1) * P],
                        yT_r[:, ko, m0:m0 + mw],
                        start=(ko == 0), stop=(ko == KO - 1))
            for ci, (m0, mw) in enumerate(kv_chunks):
                nc.vector.tensor_copy(kT[:, co, m0:m0 + mw], psk[:, ci, :mw])

        def a1_qproj(co, wq_cur):
            """qT[:, co] <- x @ Wq[:, co-block], both query halves."""
            psq = psS.tile([P, 2, 512], F32, tag="ps2")
            for ko in range(KO):
                for nn2 in range(2):
                    nc.tensor.matmul(
                        psq[:, nn2], wq_cur[:, ko],
                        xT_r[:, ko, nn2 * 512:(nn2 + 1) * 512],
                        start=(ko == 0), stop=(ko == KO - 1))
            nc.vector.tensor_copy(qT[:, co, :], psq[:, :, :])

        def a3_vproj():
            """v = y @ Wv, m-major, y-stationary, c4-paired (N=512)."""
            for mo in range(mo_n):
                psv = psS.tile([P, 2, 512], F32, tag="ps2")
                for ko in range(KO):
                    for cp in range(2):
                        nc.tensor.matmul(
                            psv[:, cp], yT_r[:, ko, mo * P:(mo + 1) * P],
                            wv_t[:, ko, cp * 512:(cp + 1) * 512],
                            start=(ko == 0), stop=(ko == KO - 1))
                nc.vector.tensor_copy(
                    vH[:, mo, :, 0:64],
                    psv[:, :, :].rearrange("p c2 (h d) -> p (c2 h) d", d=64))

        def scores_exp(co, nn2):
            """Row-tiled scores for head pair (2co, 2co+1) + exp."""
            ex = pbe.tile([P, mo_n, 2, 512], BF, tag="expS", bufs=4)
            for c in range(mo_n):
                pss = psS.tile([P, 2, 512], F32, tag="ps2")
                nc.tensor.matmul(
                    pss[:, 0], kT[0:64, co, c * P:(c + 1) * P],
                    qT[0:64, co, nn2 * 512:(nn2 + 1) * 512],
                    start=True, stop=True)
                nc.tensor.matmul(
                    pss[:, 1], kT[64:128, co, c * P:(c + 1) * P],
                    qT[64:128, co, nn2 * 512:(nn2 + 1) * 512],
                    start=True, stop=True)
                nc.scalar.activation(
                    ex[:, c], pss[:, :, :],
                    mybir.ActivationFunctionType.Exp, scale=float(SCALE))
            return ex

        def attnv_norm(co, exs, nns=(0, 1)):
            """attn @ v + softmax normalize for the pair's instances."""
            for nn2 in nns:
                ex = exs[nn2]
                for h01 in range(2):
                    h = 2 * co + h01
                    ops = psV.tile([P, 512], F32, tag="psv")
                    for c in range(mo_n):
                        nc.tensor.matmul(
                            ops[0:65], vS[:, c, h * 65:(h + 1) * 65],
                            ex[:, c, h01],
                            start=(c == 0), stop=(c == mo_n - 1))
                    den = pbe.tile([1, 512], F32, tag="den")
                    nc.vector.tensor_copy(den[:], ops[64:65])
                    rec = pbe.tile([1, 512], F32, tag="rec")
                    # approx recip must read SBUF, not PSUM (probed on HW)
                    nc.vector.reciprocal_approx_fast(rec[:], den[:])
                    bc = pbe.tile([64, 512], F32, tag="bc")
                    nc.gpsimd.partition_broadcast(bc[:], rec[:])
                    nc.vector.tensor_mul(
                        attnT[h01 * 64:h01 * 64 + 64, co,
                              nn2 * 512:(nn2 + 1) * 512],
                        ops[0:64], bc[:])

        # ---- fused schedule; A2/A1 run one slot ahead of scores so the
        # scores ldweights never wait on the just-issued qT/kT evacuation.
        wk_tiles = {0: wk_t}

        def a2_sched(co):
            """Run A2(co), prefetching the wk chunk for co+2 first."""
            cq = co // 2
            if co % 2 == 0 and cq + 1 < 4:
                wk_nxt = wstg.tile([P, KO, 256], BF, tag="wk")
                nc.sync.dma_start(wk_nxt[:],
                                  wk3[:, :, (cq + 1) * 256:(cq + 2) * 256])
                wk_tiles[cq + 1] = wk_nxt
            a2_kproj(co, wk_tiles[cq])

        def a1_sched(co):
            nonlocal wq_t
            if co + 1 < KO:
                wq_nxt = wstg.tile([P, KO, P], BF, tag="wq")
                nc.sync.dma_start(wq_nxt[:],
                                  wq3[:, :, (co + 1) * P:(co + 2) * P])
            a1_qproj(co, wq_t)
            if co + 1 < KO:
                wq_t = wq_nxt

        a2_sched(0)

        # rest of the initial loads, issued after A2(0)'s emission so its
        # DMA-completion watermark only covers y+wk0
        nc.scalar.dma_start(kc[:], keepc.rearrange("(mo p) -> p mo", p=P))
        nc.scalar.dma_start(bT[:], bp.rearrange("(o p) -> p o", p=P))
        wq_t = wstg.tile([P, KO, P], BF, tag="wq")
        nc.sync.dma_start(wq_t[:], wq3[:, :, 0:P])
        for ko in range(KO):
            nc.gpsimd.dma_start(xT_r[:, ko], xr3[:, ko])
        for i in range(4):
            nc.sync.dma_start(wv_t[:, :, i * 256:(i + 1) * 256],
                              wv3[:, :, i * 256:(i + 1) * 256])
        vH = vS.rearrange("p mo (h s) -> p mo h s", s=65)
        for mo in range(mo_n):
            nc.gpsimd.tensor_copy(vH[:, mo, :, 64],
                                  kc[:, mo:mo + 1].to_broadcast([P, H]))

        a2_sched(1)
        a1_sched(0)
        prev = None
        for co in range(KO):
            if co + 2 < KO:
                a2_sched(co + 2)
            if co + 1 < KO:
                a1_sched(co + 1)
            if co >= 4:                      # stream Wproj under B's shadow
                cw = co - 4
                nc.sync.dma_start(wp_r[:, :, cw * 256:(cw + 1) * 256],
                                  wp3[:, :, cw * 256:(cw + 1) * 256])
            exs = (scores_exp(co, 0), scores_exp(co, 1))
            if co == 0:
                a3_vproj()
            if prev is not None:
                attnv_norm(prev[0], prev[1])
            prev = (co, exs)

        def c_proj(nn2):
            """outT[c2, nn-half] = Wproj^T @ attnT + bias."""
            for c2o in range(KO):
                psc = psV.tile([P, 512], F32, tag="psv", name="psc")
                for co in range(KO):
                    nc.tensor.matmul(
                        psc[:], wp_r[:, co, c2o * P:(c2o + 1) * P],
                        attnT[:, co, nn2 * 512:(nn2 + 1) * 512],
                        start=(co == 0), stop=(co == KO - 1))
                osb = outp.tile([P, 512], F32, tag="osb")
                nc.vector.tensor_scalar_add(osb[:], psc[:],
                                            bT[:, c2o:c2o + 1])
                nc.sync.dma_start(
                    out[c2o * P:(c2o + 1) * P, nn2 * 512:(nn2 + 1) * 512],
                    osb[:])

        # last pair's nn0 half first, so C(nn0) overlaps att(7, nn1)+norms
        attnv_norm(prev[0], prev[1], nns=(0,))
        c_proj(0)
        attnv_norm(prev[0], prev[1], nns=(1,))
        c_proj(1)

    nc.finalize()
    return nc


_NC = {}


def kernel(x, y, pad_mask, Wq, Wkv, Wproj, bproj):
    x = np.asarray(x, dtype=np.float32)
    y = np.asarray(y, dtype=np.float32)
    pad_mask = np.asarray(pad_mask)
    Wq = np.asarray(Wq, dtype=np.float32)
    Wkv = np.asarray(Wkv, dtype=np.float32)
    Wproj = np.asarray(Wproj, dtype=np.float32)
    bproj = np.asarray(bproj, dtype=np.float32)

    Wqb = np.ascontiguousarray(Wq.astype(NPBF))
    Wkb = np.ascontiguousarray(Wkv[:, :DIM].astype(NPBF))
    Wvb = np.ascontiguousarray(Wkv[:, DIM:].astype(NPBF))
    Wpb = np.ascontiguousarray(Wproj.astype(NPBF))

    # compact kv: gather kept rows per batch, pad with zeros to m2c
    keep_idx = [np.nonzero(pad_mask[b] != 0)[0] for b in range(B)]
    max_kept = max(len(i) for i in keep_idx)
    m2c = M2C if max_kept <= M2C else N2
    yc = np.zeros((B, m2c, DIM), dtype=np.float32)
    keepc = np.zeros((B, m2c), dtype=NPBF)
    for b in range(B):
        k = len(keep_idx[b])
        yc[b, :k] = y[b][keep_idx[b]]
        keepc[b, :k] = 1.0

    xTb = [np.ascontiguousarray(x[b, half * R:(half + 1) * R, :].T.astype(NPBF))
           for b in range(B) for half in range(2)]
    yTb = [np.ascontiguousarray(yc[b].T.astype(NPBF)) for b in range(B)]

    in_maps = []
    for c in range(NCORES):
        b, half = c // 2, c % 2
        in_maps.append({
            "xT": xTb[c],
            "yT": yTb[b],
            "wq": Wqb, "wk": Wkb, "wv": Wvb, "wp": Wpb,
            "keepc": keepc[b],
            "bp": bproj,
        })

    if m2c not in _NC:
        _NC[m2c] = build_kernel(m2c)

    res = run_bass_kernel_spmd(_NC[m2c], in_maps, core_ids=list(range(NCORES)),
                               trace=TRACE)
    if TRACE:
        kernel.last_results = res

    full = np.empty((B, N, DIM), dtype=np.float32)
    for c in range(NCORES):
        b, half = c // 2, c % 2
        full[b, half * R:(half + 1) * R, :] = res.results[c]["out"].T
    return full


# revision 13
# speedup vs baseline: 1.0677x; 1.0050x over previous
"""CrossAttention TRN2 Bass kernel — 8-core data-parallel (batch x query-half).

Sharding: core c -> batch b=c//2, query rows [(c%2)*1024, (c%2+1)*1024).
Each core computes its 1024 output rows end-to-end (kv recomputed per
core-pair; no collectives). Host pre-transposes activations so every
matmul operand is contraction-major in DRAM.

kv compaction: masked kv positions are gathered out on the host (pad to
M2C=640 rows of zeros). Zero k rows give exp(0)=1 at pads, but the
ones-column appended to v carries the keep flag, so pads contribute
exactly 0 to both the attention numerator and the softmax denominator.

v2 restructure vs the 290us baseline:
- Scores matmuls for a head PAIR run row-tiled (64x128 mode, tiles
  T0/T8): head 2co lives on SBUF partitions 0-63, head 2co+1 on 64-127
  (the kT/qT layout already interleaves heads that way), so the two
  64-contraction matmuls execute CONCURRENTLY in the two array halves.
- One fused schedule: per co-slot [A2 kproj(co), A1 qproj(co),
  scores+exp(co), attnv(co-1)] so the ACT-engine exp stream (~11us/pair)
  hides under PE work of the same slot; vproj runs up front with
  y-stationary c4-paired matmuls (80 mms instead of 160).
- exp granularity: one ACT op per m-chunk covering BOTH heads of the
  pair ([P,2,512] PSUM -> [P,2,512] bf16), so PSUM in scores stays at
  3x2 banks and ACT per-op fixed cost stays amortized.
- den handled by the v ones-column as before; den copy moved off ACT
  (vector), normalize mul on vector, partition broadcast on gpsimd.
"""

import sys

sys.path.insert(0, "/opt/trn_rl_repo")

from contextlib import ExitStack

import ml_dtypes
import numpy as np

import concourse.bass as bass
import concourse.tile as tile
from concourse import bacc, mybir
from concourse.bass_utils import run_bass_kernel_spmd

B, N, N2 = 4, 2048, 1024
DIM, H, HD = 1024, 16, 64
SCALE = HD ** -0.5
P = 128
R = 1024          # query rows per core
NCORES = 8
KO = DIM // P     # 8 contraction chunks
F32 = mybir.dt.float32
BF = mybir.dt.bfloat16
NPBF = ml_dtypes.bfloat16

M2C = 640         # compacted kv length (5 x 128); kept count must fit

TRACE = False


def build_kernel(m2c=M2C):
    mo_n = m2c // P
    nc = bacc.Bacc("TRN2", target_bir_lowering=False, debug=False,
                   num_devices=NCORES)
    xT = nc.dram_tensor("xT", [DIM, R], BF, kind="ExternalInput").ap()
    yT = nc.dram_tensor("yT", [DIM, m2c], BF, kind="ExternalInput").ap()
    wq = nc.dram_tensor("wq", [DIM, DIM], BF, kind="ExternalInput").ap()
    wk = nc.dram_tensor("wk", [DIM, DIM], BF, kind="ExternalInput").ap()
    wv = nc.dram_tensor("wv", [DIM, DIM], BF, kind="ExternalInput").ap()
    wp = nc.dram_tensor("wp", [DIM, DIM], BF, kind="ExternalInput").ap()
    keepc = nc.dram_tensor("keepc", [m2c], BF, kind="ExternalInput").ap()
    bp = nc.dram_tensor("bp", [DIM], F32, kind="ExternalInput").ap()
    out = nc.dram_tensor("out", [DIM, R], F32, kind="ExternalOutput").ap()

    # kv free-dim chunking for the k^T projection (<=512 per PSUM bank)
    kv_chunks = [(i, min(512, m2c - i)) for i in range(0, m2c, 512)]
    assert len(kv_chunks) <= 2

    with tile.TileContext(nc, pool_alloc_mode="queue") as tc, ExitStack() as ctx:
        persist = ctx.enter_context(tc.tile_pool(name="persist", bufs=1))
        qT = persist.tile([P, KO, R], BF)           # q^T, c-major
        kT = persist.tile([P, KO, m2c], BF)         # k^T, c-major
        vS = persist.tile([P, mo_n, H * 65], BF)    # v[m,c] + keep col / head
        attnT = persist.tile([P, KO, R], BF)        # attn out^T, c-major
        wp_r = persist.tile([P, KO, DIM], BF)       # Wproj, loaded during B
        kc = persist.tile([P, mo_n], BF)            # keep col, m-major
        bT = persist.tile([P, KO], F32)
        xT_r = [persist.tile([P, R], BF, name=f"xk{ko}") for ko in range(KO)]
        yT_r = [persist.tile([P, m2c], BF, name=f"yk{ko}") for ko in range(KO)]
        wv_t = persist.tile([P, KO, DIM], BF)       # full Wv staged

        wq3 = wq.rearrange("(ko p) c -> p ko c", p=P)
        wk3 = wk.rearrange("(ko p) c -> p ko c", p=P)
        wv3 = wv.rearrange("(ko p) c -> p ko c", p=P)
        wp3 = wp.rearrange("(ko p) c -> p ko c", p=P)
        xr3 = xT.rearrange("(ko p) f -> p ko f", p=P)
        yr3 = yT.rearrange("(ko p) f -> p ko f", p=P)

        wstg = ctx.enter_context(tc.tile_pool(name="wstg", bufs=2))
        pbe = ctx.enter_context(tc.tile_pool(name="pbe", bufs=2))
        outp = ctx.enter_context(tc.tile_pool(name="outp", bufs=3))
        # PSUM: ps2 3x[P,2,512] (6 banks) + psv 2x[P,512] (2 banks) = 8
        psS = ctx.enter_context(tc.tile_pool(name="psS", bufs=3, space="PSUM"))
        psV = ctx.enter_context(tc.tile_pool(name="psV", bufs=2, space="PSUM"))

        # ---- initial DMA issues, ordered by when compute needs them:
        # y+wk0 gate A2(0) (~5us), x+wq0 gate A1(0) (~12us), wv gates A3.
        # Only A2(0)'s inputs are issued before its emission so its DMA
        # watermark stays low (deps are per-queue counters).
        for ko in range(KO):
            nc.scalar.dma_start(yT_r[ko][:], yr3[:, ko])
        wk_t = wstg.tile([P, KO, 256], BF, tag="wk")
        nc.sync.dma_start(wk_t[:], wk3[:, :, 0:256])

        # PE warmup: ~5us of throwaway matmuls so the tensor clock is at
        # max p-state by the time the real work lands.
        warm = persist.tile([P, 512], BF)
        nc.vector.memset(warm[:], 0)
        psw = psV.tile([P, 512], F32, tag="psv", name="psw")
        for i in range(24):
            nc.tensor.matmul(psw[0:64, :], warm[:, 0:64], warm[:, :],
                             start=True, stop=True)

        def a2_kproj(co, wk_cur):
            """kT[:, co] <- Wk[:, co-block]^T @ y^T (8 ko accumulation)."""
            c2 = co % 2
            psk = psS.tile([P, 2, 512], F32, tag="ps2")
            for ko in range(KO):
                for ci, (m0, mw) in enumerate(kv_chunks):
                    nc.tensor.matmul(
                        psk[:, ci, :mw], wk_cur[:, ko, c2 * P:(c2 + 1) * P],
                        yT_r[ko][:, m0:m0 + mw],
                        start=(ko == 0), stop=(ko == KO - 1))
            for ci, (m0, mw) in enumerate(kv_chunks):
                nc.vector.tensor_copy(kT[:, co, m0:m0 + mw], psk[:, ci, :mw])

        def a1_qproj(co, wq_cur):
            """qT[:, co] <- x @ Wq[:, co-block], both query halves."""
            psq = psS.tile([P, 2, 512], F32, tag="ps2")
            for ko in range(KO):
                for nn2 in range(2):
                    nc.tensor.matmul(
                        psq[:, nn2], wq_cur[:, ko],
                        xT_r[ko][:, nn2 * 512:(nn2 + 1) * 512],
                        start=(ko == 0), stop=(ko == KO - 1))
            nc.vector.tensor_copy(qT[:, co, :], psq[:, :, :])

        def a3_vproj():
            """v = y @ Wv, m-major, y-stationary, c4-paired (N=512)."""
            for mo in range(mo_n):
                psv = psS.tile([P, 2, 512], F32, tag="ps2")
                for ko in range(KO):
                    for cp in range(2):
                        nc.tensor.matmul(
                            psv[:, cp], yT_r[ko][:, mo * P:(mo + 1) * P],
                            wv_t[:, ko, cp * 512:(cp + 1) * 512],
                            start=(ko == 0), stop=(ko == KO - 1))
                nc.vector.tensor_copy(
                    vH[:, mo, :, 0:64],
                    psv[:, :, :].rearrange("p c2 (h d) -> p (c2 h) d", d=64))

        def scores_exp(co, nn2):
            """Row-tiled scores for head pair (2co, 2co+1) + exp."""
            ex = pbe.tile([P, mo_n, 2, 512], BF, tag="expS", bufs=4)
            for c in range(mo_n):
                pss = psS.tile([P, 2, 512], F32, tag="ps2")
                nc.tensor.matmul(
                    pss[:, 0], kT[0:64, co, c * P:(c + 1) * P],
                    qT[0:64, co, nn2 * 512:(nn2 + 1) * 512],
                    start=True, stop=True)
                nc.tensor.matmul(
                    pss[:, 1], kT[64:128, co, c * P:(c + 1) * P],
                    qT[64:128, co, nn2 * 512:(nn2 + 1) * 512],
                    start=True, stop=True)
                nc.scalar.activation(
                    ex[:, c], pss[:, :, :],
                    mybir.ActivationFunctionType.Exp, scale=float(SCALE))
            return ex

        def attnv_norm(co, exs, nns=(0, 1)):
            """attn @ v + softmax normalize for the pair's instances."""
            for nn2 in nns:
                ex = exs[nn2]
                for h01 in range(2):
                    h = 2 * co + h01
                    ops = psV.tile([P, 512], F32, tag="psv")
                    for c in range(mo_n):
                        nc.tensor.matmul(
                            ops[0:65], vS[:, c, h * 65:(h + 1) * 65],
                            ex[:, c, h01],
                            start=(c == 0), stop=(c == mo_n - 1))
                    den = pbe.tile([1, 512], F32, tag="den")
                    nc.vector.tensor_copy(den[:], ops[64:65])
                    rec = pbe.tile([1, 512], F32, tag="rec")
                    # approx recip must read SBUF, not PSUM (probed on HW)
                    nc.vector.reciprocal_approx_fast(rec[:], den[:])
                    bc = pbe.tile([64, 512], F32, tag="bc")
                    nc.gpsimd.partition_broadcast(bc[:], rec[:])
                    nc.vector.tensor_mul(
                        attnT[h01 * 64:h01 * 64 + 64, co,
                              nn2 * 512:(nn2 + 1) * 512],
                        ops[0:64], bc[:])

        # ---- fused schedule; A2/A1 run one slot ahead of scores so the
        # scores ldweights never wait on the just-issued qT/kT evacuation.
        wk_tiles = {0: wk_t}

        def a2_sched(co):
            """Run A2(co), prefetching the wk chunk for co+2 first."""
            cq = co // 2
            if co % 2 == 0 and cq + 1 < 4:
                wk_nxt = wstg.tile([P, KO, 256], BF, tag="wk")
                nc.sync.dma_start(wk_nxt[:],
                                  wk3[:, :, (cq + 1) * 256:(cq + 2) * 256])
                wk_tiles[cq + 1] = wk_nxt
            a2_kproj(co, wk_tiles[cq])

        def a1_sched(co):
            nonlocal wq_t
            if co + 1 < KO:
                wq_nxt = wstg.tile([P, KO, P], BF, tag="wq")
                nc.sync.dma_start(wq_nxt[:],
                                  wq3[:, :, (co + 1) * P:(co + 2) * P])
            a1_qproj(co, wq_t)
            if co + 1 < KO:
                wq_t = wq_nxt

        a2_sched(0)

        # rest of the initial loads, issued after A2(0)'s emission so its
        # DMA-completion watermark only covers y+wk0
        nc.scalar.dma_start(kc[:], keepc.rearrange("(mo p) -> p mo", p=P))
        nc.scalar.dma_start(bT[:], bp.rearrange("(o p) -> p o", p=P))
        wq_t = wstg.tile([P, KO, P], BF, tag="wq")
        nc.sync.dma_start(wq_t[:], wq3[:, :, 0:P])
        for ko in range(KO):
            nc.gpsimd.dma_start(xT_r[ko][:], xr3[:, ko])
        for i in range(4):
            nc.sync.dma_start(wv_t[:, :, i * 256:(i + 1) * 256],
                              wv3[:, :, i * 256:(i + 1) * 256])
        vH = vS.rearrange("p mo (h s) -> p mo h s", s=65)
        for mo in range(mo_n):
            nc.gpsimd.tensor_copy(vH[:, mo, :, 64],
                                  kc[:, mo:mo + 1].to_broadcast([P, H]))

        a2_sched(1)
        a1_sched(0)
        prev = None
        for co in range(KO):
            if co + 2 < KO:
                a2_sched(co + 2)
            if co + 1 < KO:
                a1_sched(co + 1)
            if co >= 4:                      # stream Wproj under B's shadow
                cw = co - 4
                nc.sync.dma_start(wp_r[:, :, cw * 256:(cw + 1) * 256],
                                  wp3[:, :, cw * 256:(cw + 1) * 256])
            exs = (scores_exp(co, 0), scores_exp(co, 1))
            if co == 0:
                a3_vproj()
            if prev is not None:
                attnv_norm(prev[0], prev[1])
            prev = (co, exs)

        def c_proj(nn2):
            """outT[c2, nn-half] = Wproj^T @ attnT + bias."""
            for c2o in range(KO):
                psc = psV.tile([P, 512], F32, tag="psv", name="psc")
                for co in range(KO):
                    nc.tensor.matmul(
                        psc[:], wp_r[:, co, c2o * P:(c2o + 1) * P],
                        attnT[:, co, nn2 * 512:(nn2 + 1) * 512],
                        start=(co == 0), stop=(co == KO - 1))
                osb = outp.tile([P, 512], F32, tag="osb")
                nc.vector.tensor_scalar_add(osb[:], psc[:],
                                            bT[:, c2o:c2o + 1])
                nc.sync.dma_start(
                    out[c2o * P:(c2o + 1) * P, nn2 * 512:(nn2 + 1) * 512],
                    osb[:])

        # last pair's nn0 half first, so C(nn0) overlaps att(7, nn1)+norms
        attnv_norm(prev[0], prev[1], nns=(0,))
        c_proj(0)
        attnv_norm(prev[0], prev[1], nns=(1,))
        c_proj(1)

    nc.finalize()
    return nc


_NC = {}


def kernel(x, y, pad_mask, Wq, Wkv, Wproj, bproj):
    x = np.asarray(x, dtype=np.float32)
    y = np.asarray(y, dtype=np.float32)
    pad_mask = np.asarray(pad_mask)
    Wq = np.asarray(Wq, dtype=np.float32)
    Wkv = np.asarray(Wkv, dtype=np.float32)
    Wproj = np.asarray(Wproj, dtype=np.float32)
    bproj = np.asarray(bproj, dtype=np.float32)

    Wqb = np.ascontiguousarray(Wq.astype(NPBF))
    Wkb = np.ascontiguousarray(Wkv[:, :DIM].astype(NPBF))
    Wvb = np.ascontiguousarray(Wkv[:, DIM:].astype(NPBF))
    Wpb = np.ascontiguousarray(Wproj.astype(NPBF))

    # compact kv: gather kept rows per batch, pad with zeros to m2c
    keep_idx = [np.nonzero(pad_mask[b] != 0)[0] for b in range(B)]
    max_kept = max(len(i) for i in keep_idx)
    m2c = M2C if max_kept <= M2C else N2
    yc = np.zeros((B, m2c, DIM), dtype=np.float32)
    keepc = np.zeros((B, m2c), dtype=NPBF)
    for b in range(B):
        k = len(keep_idx[b])
        yc[b, :k] = y[b][keep_idx[b]]
        keepc[b, :k] = 1.0

    xTb = [np.ascontiguousarray(x[b, half * R:(half + 1) * R, :].T.astype(NPBF))
           for b in range(B) for half in range(2)]
    yTb = [np.ascontiguousarray(yc[b].T.astype(NPBF)) for b in range(B)]

    in_maps = []
    for c in range(NCORES):
        b, half = c // 2, c % 2
        in_maps.append({
            "xT": xTb[c],
            "yT": yTb[b],
            "wq": Wqb, "wk": Wkb, "wv": Wvb, "wp": Wpb,
            "keepc": keepc[b],
            "bp": bproj,
        })

    if m2c not in _NC:
        _NC[m2c] = build_kernel(m2c)

    res = run_bass_kernel_spmd(_NC[m2c], in_maps, core_ids=list(range(NCORES)),
                               trace=TRACE)
    if TRACE:
        kernel.last_results = res

    full = np.empty((B, N, DIM), dtype=np.float32)
    for c in range(NCORES):
        b, half = c // 2, c % 2
        full[b, half * R:(half + 1) * R, :] = res.results[c]["out"].T
    return full
